# revision 1
# baseline (speedup 1.0000x reference)
"""AttentionHead kernel for 8 Trainium2 NeuronCores (SPMD data-parallel).

Problem: q/k/v projections [1024->64] + masked softmax attention,
B=4, S=2048, d_model=1024, d_k=64.

Sharding: 8 cores = 4 batches x 2 query-halves. Each core handles one
(batch, q-half): query shard [1024, 1024], full key/value for its batch
[2048, 1024], mask shard [1024, 2048]. Weights replicated.

Per-core device pipeline (everything contracts on the partition dim;
all inputs host-packed so each DMA is one large contiguous transfer):
  - projections: qT [64, sq], kT [64, skv] via matmul(lhsT=w_t, rhs=xT)
  - v projected per skv-block to natural [128, 64] (lhsT=valueT block),
    augmented with a ones column
  - scores computed TRANSPOSED [skv_tile=128, sq=1024]: no probability
    transpose needed anywhere
  - ACT exp reads scores straight from PSUM with the 1024**-0.5 scale
    fused; masked positions then set to 1.0 (==exp(1e-9) in fp32) via
    copy_predicated on the bf16 E tile
  - PV accumulates transposed: oT[65, sq] += vaug_j.T @ E_j; row 64 is
    the softmax denominator (free via the ones column)
  - finalize: PE-transpose oT back to [sq, 65], multiply by reciprocal
    of column 64, one batched output DMA
"""

import numpy as np
import ml_dtypes

B = 4
S = 2048
D_MODEL = 1024
D_K = 64
N_CORES = 8

P = 128
SQ = S // 2          # per-core query rows (1024)
SKV = S              # per-core kv rows (2048)
MB = D_MODEL // P    # 8 m-blocks (contraction)
JT = SKV // P        # 16 skv tiles
IT = SQ // P         # 8 sq tiles
NQC = SQ // 512      # 2 q chunks
NKC = SKV // 512     # 4 k chunks
VG = 4               # v/mask DMA groups
JPG = JT // VG       # skv tiles per DMA group

_BF16 = ml_dtypes.bfloat16

_cached_nc = None


def _build_nc():
    import concourse.mybir as mybir
    import concourse.tile as tile
    from concourse import bacc

    bf16 = mybir.dt.bfloat16
    f32 = mybir.dt.float32
    u8 = mybir.dt.uint8

    nc = bacc.Bacc(None, target_bir_lowering=False)

    w_d = nc.dram_tensor("w_all", [P, MB, 3 * D_K], bf16, kind="ExternalInput")
    q_d = nc.dram_tensor("q_t", [P, MB, SQ], bf16, kind="ExternalInput")
    k_d = nc.dram_tensor("k_t", [P, MB, SKV], bf16, kind="ExternalInput")
    v_d = nc.dram_tensor("v_t", [P, JT, MB, P], bf16, kind="ExternalInput")
    m_d = nc.dram_tensor("mask_t", [P, JT, SQ], u8, kind="ExternalInput")
    idn_d = nc.dram_tensor("idn_t", [D_K + 1, D_K + 1], f32, kind="ExternalInput")
    out_d = nc.dram_tensor("out", [P, IT, D_K], f32, kind="ExternalOutput")

    with tile.TileContext(nc) as tc:
        with (
            tc.tile_pool(name="const", bufs=1) as cpool,
            tc.tile_pool(name="inp", bufs=1) as ipool,
            tc.tile_pool(name="proj", bufs=1) as jpool,
            tc.tile_pool(name="fin", bufs=2) as fpool,
            tc.tile_pool(name="ps_pqk", bufs=1, space="PSUM") as ps_pqk,
            tc.tile_pool(name="ps_pv", bufs=2, space="PSUM") as ps_pv,
            tc.tile_pool(name="ps_s", bufs=3, space="PSUM") as ps_s,
            tc.tile_pool(name="ps_o", bufs=1, space="PSUM") as ps_o,
        ):
            # ---- input DMAs: few, large, streamed in compute order ----
            w_sb = cpool.tile([P, MB, 3 * D_K], bf16, tag="w")
            nc.sync.dma_start(out=w_sb, in_=w_d[:])
            q_sb = ipool.tile([P, MB, SQ], bf16, tag="q")
            nc.sync.dma_start(out=q_sb, in_=q_d[:])

            kcs = [None] * NKC
            mgs = [None] * VG
            vgs = [None] * VG

            def dma_k(t):
                kc = ipool.tile([P, MB, 512], bf16, tag=f"k{t}", name=f"k{t}")
                nc.sync.dma_start(out=kc, in_=k_d[:, :, t * 512 : (t + 1) * 512])
                kcs[t] = kc

            def dma_m(g):
                mg = ipool.tile([P, JPG, SQ], u8, tag=f"m{g}", name=f"m{g}")
                nc.sync.dma_start(out=mg, in_=m_d[:, g * JPG : (g + 1) * JPG, :])
                mgs[g] = mg

            def dma_v(g):
                vg = ipool.tile([P, JPG, MB, P], bf16, tag=f"v{g}", name=f"v{g}")
                nc.sync.dma_start(out=vg, in_=v_d[:, g * JPG : (g + 1) * JPG, :, :])
                vgs[g] = vg

            # stream order tuned to dependency-ready times
            dma_k(0)
            dma_m(0)
            dma_k(1)
            dma_v(0)
            dma_k(2)
            dma_m(1)
            dma_v(1)
            dma_k(3)
            dma_m(2)
            dma_m(3)
            dma_v(2)
            dma_v(3)

            def wq(i):
                return w_sb[:, i, 0:D_K]

            def wk(i):
                return w_sb[:, i, D_K : 2 * D_K]

            def wv(i):
                return w_sb[:, i, 2 * D_K : 3 * D_K]

            # ---- constants ----
            ones_bf = cpool.tile([P, 2, SQ], bf16, tag="ones")
            nc.vector.memset(ones_bf, 1.0)
            idn = cpool.tile([D_K + 1, D_K + 1], f32, tag="idn")
            nc.sync.dma_start(out=idn, in_=idn_d[:])

            # ---- PE warm-up: keep the HAM clock at 2.4 GHz before real
            # matmuls start (burns idle PE time while DMAs stream) ----
            warm_ps = ps_pqk.tile([D_K, 512], f32, tag="pqk", name="warm")
            w_flat = w_sb.rearrange("p mb k -> p (mb k)")
            for wi in range(16):
                nc.tensor.matmul(
                    warm_ps,
                    lhsT=wq(0),
                    rhs=w_flat[:, 0:512],
                    start=(wi == 0),
                    stop=(wi == 15),
                )

            # ---- q/k projections ----
            qTd = jpool.tile([D_K, SQ], bf16, tag="qT")
            kTd = jpool.tile([D_K, SKV], bf16, tag="kT")

            def kproj(t):
                pp = ps_pqk.tile([D_K, 512], f32, tag="pqk", name=f"pk{t}")
                for i in range(MB):
                    nc.tensor.matmul(
                        pp,
                        lhsT=wk(i),
                        rhs=kcs[t][:, i, :],
                        start=(i == 0),
                        stop=(i == MB - 1),
                    )
                sl = slice(t * 512, (t + 1) * 512)
                nc.vector.tensor_copy(kTd[:, sl], pp)

            for t in range(NQC):
                pp = ps_pqk.tile([D_K, 512], f32, tag="pqk", name=f"pq{t}")
                for i in range(MB):
                    nc.tensor.matmul(
                        pp,
                        lhsT=wq(i),
                        rhs=q_sb[:, i, t * 512 : (t + 1) * 512],
                        start=(i == 0),
                        stop=(i == MB - 1),
                    )
                sl = slice(t * 512, (t + 1) * 512)
                nc.vector.tensor_copy(qTd[:, sl], pp)
            kproj(0)

            # ---- per-j v-aug tiles (col 64 = ones) ----
            vaugs = []
            for j in range(JT):
                va = jpool.tile([P, D_K + 1], bf16, tag=f"va{j}", name=f"va{j}")
                nc.vector.memset(va[:, D_K : D_K + 1], 1.0)
                vaugs.append(va)

            # ---- per-pair E tiles [128, 2, 1024] ----
            Eps = [
                jpool.tile([P, 2, SQ], bf16, tag=f"E{p}", name=f"E{p}")
                for p in range(JT // 2)
            ]

            def E_of(j2):
                return Eps[j2 // 2][:, j2 % 2, :]

            # ---- transposed output accumulator [65, 1024] f32 = 2 banks,
            # one accumulation group per bank ----
            oTp = ps_o.tile([D_K + 1, SQ], f32, tag="oT")

            # ---- main pipeline over skv tiles.
            # Emission order is engine-queue order. Cross-engine consumers
            # are emitted a few iterations late (vproj @ j-2, PV @ j-4) so
            # the in-order PE stream never stalls on the DVE/ACT chain.
            pvs = [None] * JT

            def emit_vproj(j2):
                g, jj = divmod(j2, JPG)
                pv = ps_pv.tile([P, D_K], f32, tag="pv", name=f"pv{j2}")
                for i in range(MB):
                    nc.tensor.matmul(
                        pv,
                        lhsT=vgs[g][:, jj, i, :],
                        rhs=wv(i),
                        start=(i == 0),
                        stop=(i == MB - 1),
                    )
                pvs[j2] = pv

            def emit_cast(j2):
                nc.vector.tensor_copy(vaugs[j2][:, 0:D_K], pvs[j2])

            def emit_pv(j2):
                for c in range(NQC):
                    nc.tensor.matmul(
                        oTp[:, c * 512 : (c + 1) * 512],
                        lhsT=vaugs[j2],
                        rhs=E_of(j2)[:, c * 512 : (c + 1) * 512],
                        start=(j2 == 0),
                        stop=(j2 == JT - 1),
                    )

            for j in range(JT):
                g, jj = divmod(j, JPG)

                # transposed scores [skv_tile 128, sq] as two concurrent
                # row-group matmuls into separate single-bank psum tiles
                spa = ps_s.tile([P, 512], f32, tag="sp", name=f"spa{j}")
                spb = ps_s.tile([P, 512], f32, tag="sp", name=f"spb{j}")
                jsl = slice(j * P, (j + 1) * P)
                nc.tensor.matmul(
                    spa, lhsT=kTd[:, jsl], rhs=qTd[:, 0:512], start=True, stop=True
                )
                nc.tensor.matmul(
                    spb, lhsT=kTd[:, jsl], rhs=qTd[:, 512:1024], start=True, stop=True
                )
                if jj == 0 and g + 1 < NKC:
                    kproj(g + 1)  # one group ahead, after this group's scores
                if j >= 2:
                    emit_vproj(j - 2)
                if j >= 4:
                    emit_pv(j - 4)

                # E = exp(s / sqrt(d_model)) straight from PSUM, cast bf16
                for half, sp in ((0, spa), (1, spb)):
                    nc.scalar.activation(
                        out=E_of(j)[:, half * 512 : (half + 1) * 512],
                        in_=sp,
                        func=mybir.ActivationFunctionType.Exp,
                        scale=float(D_MODEL) ** -0.5,
                    )
                if j >= 3:
                    emit_cast(j - 3)
                if j % 2 == 1:
                    # masked positions -> 1.0 (== exp(1e-9) in fp32);
                    # one DVE op per pair of skv tiles
                    nc.vector.copy_predicated(
                        out=Eps[j // 2],
                        mask=mgs[g][:, jj - 1 : jj + 1, :],
                        data=ones_bf,
                    )

            for j2 in (JT - 2, JT - 1):
                emit_vproj(j2)
            for j2 in (JT - 3, JT - 2, JT - 1):
                emit_cast(j2)
            for j2 in range(JT - 4, JT):
                emit_pv(j2)

            # ---- finalize: transpose oT back (2 blocks per psum bank),
            # batched reciprocals, divide by ones-row ----
            oT_sb = jpool.tile([D_K + 1, SQ], f32, tag="oTs")
            nc.vector.tensor_copy(oT_sb, oTp)
            ob = fpool.tile([P, IT, D_K], f32, tag="ob", bufs=1)
            tps = []
            for t in range(IT // 2):
                tp = ps_s.tile([P, 2, D_K + 1], f32, tag="sp", name=f"tp{t}")
                for h in range(2):
                    i = 2 * t + h
                    nc.tensor.transpose(
                        tp[:, h, :], in_=oT_sb[:, i * P : (i + 1) * P], identity=idn
                    )
                tps.append(tp)
            for t in range(IT // 2):
                r2 = fpool.tile([P, 2], f32, tag="r")
                nc.vector.reciprocal(r2, tps[t][:, :, D_K])
                for h in range(2):
                    i = 2 * t + h
                    nc.vector.tensor_scalar_mul(
                        ob[:, i, :], tps[t][:, h, 0:D_K], r2[:, h : h + 1]
                    )
            nc.sync.dma_start(out=out_d[:], in_=ob)

    nc.finalize()
    return nc


def _get_nc():
    global _cached_nc
    if _cached_nc is None:
        _cached_nc = _build_nc()
    return _cached_nc


def _pack_mb(x_t):
    """[D_MODEL, s] -> [128, MB, s] (m-block packed, contiguous)."""
    s = x_t.shape[1]
    return np.ascontiguousarray(x_t.reshape(MB, P, s).transpose(1, 0, 2))


def _shard_inputs(query, key, value, mask, w_q, w_k, w_v):
    """Host-side shard + layout prep. Core c -> (batch c//2, q-half c%2)."""
    w_all = np.concatenate(
        [
            w.T.astype(_BF16).reshape(MB, P, D_K).transpose(1, 0, 2)
            for w in (w_q, w_k, w_v)
        ],
        axis=2,
    )
    w_all = np.ascontiguousarray(w_all)
    in_maps = []
    for c in range(N_CORES):
        b, h = divmod(c, 2)
        s0 = h * SQ
        q_t = query[b, s0 : s0 + SQ, :].T.astype(_BF16)
        k_t = key[b].T.astype(_BF16)
        v_t = value[b].T.astype(_BF16)
        m_t = mask[b, s0 : s0 + SQ, :].T.astype(np.uint8)
        in_maps.append(
            {
                "w_all": w_all,
                "q_t": _pack_mb(q_t),
                "k_t": _pack_mb(k_t),
                # [m, skv] -> [128, JT, MB, 128]: [p][j][i][s']
                "v_t": np.ascontiguousarray(
                    v_t.reshape(MB, P, JT, P).transpose(1, 2, 0, 3)
                ),
                # [skv, sq] -> [128, JT, SQ]
                "mask_t": np.ascontiguousarray(
                    m_t.reshape(JT, P, SQ).transpose(1, 0, 2)
                ),
                "idn_t": np.eye(D_K + 1, dtype=np.float32),
            }
        )
    return in_maps


def run(inputs, trace=False):
    """Run the SPMD kernel; returns (output [B,S,D_K] f32, BassKernelResults)."""
    from concourse.bass_utils import run_bass_kernel_spmd

    nc = _get_nc()
    in_maps = _shard_inputs(**inputs)
    res = run_bass_kernel_spmd(
        nc, in_maps, core_ids=list(range(N_CORES)), trace=trace
    )
    out = np.empty((B, S, D_K), np.float32)
    for c in range(N_CORES):
        b, h = divmod(c, 2)
        # device out is [128, IT, 64]: row = i*128+p
        o = res.results[c]["out"].transpose(1, 0, 2).reshape(SQ, D_K)
        out[b, h * SQ : (h + 1) * SQ, :] = o
    return out, res


def kernel(**inputs):
    out, _ = run(inputs, trace=False)
    return out

